# revision 55
# baseline (speedup 1.0000x reference)
"""Trainium2 Bass kernel for the low-rank linear operator.

Math: the reference collapses algebraically. With y = linspace(-1,1,H),
x = linspace(-1,1,W), dx = 2/(W-1):

  Vy[b,i] = sum_{h,w} v[b,i,h,w] * y_h
  Vx[b,i] = sum_{h,w} v[b,i,h,w] * x_w
  inner[b,r] = dx * sum_i (Vy[b,i]*psi[r,i,0] + Vx[b,i]*psi[r,i,1])
  A[b,o] = sum_r inner[b,r]*phi[o,r,0];  Bc[b,o] = sum_r inner[b,r]*phi[o,r,1]
  u[b,o,h,w] = A[b,o]*y_h + Bc[b,o]*x_w

Sharding: data-parallel over batch, 2 batches per core, 8 cores, no
collectives.

The kernel is HBM-bandwidth bound (read v + write u). All HBM traffic is
bf16 (host casts in/out; rel-err budget 2e-2 vs ~0.5% incurred), halving
bytes vs f32. Layout: four h-rows per partition, p = 64*(i%2) + h//4, so
every DMA descriptor moves 2KB contiguous even at 2B/elem.

Reduction: PE matmul with a block-diagonal [128,4] stationary reduces the
h-quad partition dim for both channel parities at once -> psum [4, 512]
rows (even-ch y-weighted, even-ch colsum, odd-ch y-weighted, odd-ch
colsum). Since y is affine, y[4q+hh] = y[4q] + hh*dy, the hh correction is
folded into a second full-width DVE mult+reduce pass after an SBUF->SBUF
DMA re-partitions the drained psum rows to [128, (hh w)]. Tiny PE matmuls
give inner -> (A,B) -> per-partition scale/bias; DVE/ACT/Pool generate u
tiles as x_w*B + (A*y) bias directly in bf16.
"""

import sys

try:
    import concourse.bass as bass  # noqa: F401
except ImportError:
    for _p in ("/opt/trn_rl_repo", "/root/.axon_site/_ro/trn_rl_repo"):
        if _p not in sys.path:
            sys.path.insert(0, _p)

import ml_dtypes
import numpy as np

import concourse.bacc as bacc
import concourse.bass as bass
import concourse.mybir as mybir
import concourse.tile as tile
from concourse.bass_utils import run_bass_kernel_spmd

F32 = mybir.dt.float32
BF16 = mybir.dt.bfloat16
MULT = mybir.AluOpType.mult
ADD = mybir.AluOpType.add
BFNP = ml_dtypes.bfloat16

B, CI, CO, R, H, W = 16, 64, 64, 64, 256, 256
N_CORES = 8
BPC = B // N_CORES  # batches per core
NBLK = 8            # input DMA blocks per batch (8 channels each)
NTG = 8             # output DMA blocks per batch (8 channels each)

# generation-engine rotation (gpsimd has no drain duty -> give it more)
_GEN_ENGINES = (
    "dve", "act", "pool", "dve", "act", "dve", "pool", "act",
    "dve", "act", "pool", "dve", "act", "dve", "pool", "act",
)


def build_nc():
    nc = bacc.Bacc("TRN2", target_bir_lowering=False, debug=False)

    v = nc.dram_tensor("v", [BPC, CI, H, W], BF16, kind="ExternalInput")
    catbf = nc.dram_tensor("catbf", [128, 4 + 3 * 4 * W + W], BF16, kind="ExternalInput")
    catf32 = nc.dram_tensor("catf32", [128, 4 * CO + 4], F32, kind="ExternalInput")
    erows = nc.dram_tensor("erows", [1, 256], F32, kind="ExternalInput")
    u = nc.dram_tensor("u", [BPC, CO, H, W], BF16, kind="ExternalOutput")

    with tile.TileContext(nc) as tc:
        with (
            tc.tile_pool(name="consts", bufs=1) as consts,
            tc.tile_pool(name="inp", bufs=1) as in_pool,
            tc.tile_pool(name="outp", bufs=3) as out_pool,
            tc.tile_pool(name="scr", bufs=1) as scratch,
            tc.tile_pool(name="sblkp", bufs=4) as sblk_pool,
            tc.tile_pool(name="s2p", bufs=1) as s2_pool,
            tc.tile_pool(name="bc", bufs=4) as bc_pool,
            tc.tile_pool(name="psumP", bufs=3, space="PSUM") as psum_p,
            tc.tile_pool(name="psumT", bufs=1, space="PSUM") as psum_t,
            tc.tile_pool(name="psumBC", bufs=1, space="PSUM") as psum_bc,
            tc.tile_pool(name="dram", bufs=2, space="DRAM") as dram_pool,
        ):
            sb_catbf = consts.tile([128, 4 + 3 * 4 * W + W], BF16)
            nc.scalar.dma_start(sb_catbf[:], catbf[:])
            sb_catf = consts.tile([128, 4 * CO + 4], F32)
            nc.scalar.dma_start(sb_catf[:], catf32[:])
            sb_erows = consts.tile([1, 256], F32)
            nc.scalar.dma_start(sb_erows[:], erows[:])
            sb_y4cat = sb_catbf[:, 0:4]
            sb_wty = sb_catbf[:, 4 : 4 + 4 * W].rearrange("p (hh w) -> p hh w", hh=4)
            sb_wtx = sb_catbf[:, 4 + 4 * W : 4 + 8 * W].rearrange(
                "p (hh w) -> p hh w", hh=4
            )
            sb_xrep = sb_catbf[:, 4 + 8 * W : 4 + 9 * W]
            sb_ybig = sb_catbf[:, 4 + 9 * W : 4 + 13 * W].rearrange(
                "p (hh w) -> p hh w", hh=4
            )
            sb_my = sb_catf[:, 0 : 2 * CO]
            sb_mx = sb_catf[:, 2 * CO : 4 * CO]
            sb_ycol4 = sb_catf[:, 4 * CO : 4 * CO + 4]
            sb_erow0 = sb_erows[:, 0:128]
            sb_erow1 = sb_erows[:, 128:256]

            def reduce_phase(b, interleave=None):
                """v[b] -> s2[b]: [128=(ihi r), (hh w)] partial sums."""
                s2 = s2_pool.tile([128, 4, W], BF16, tag=f"s2{b}")
                gy = scratch.tile([128, 1], F32, tag=f"gy{b}")
                gx = scratch.tile([128, 1], F32, tag=f"gx{b}")
                dscr = dram_pool.tile([32, 4, 2, 512], BF16, tag="dscr")
                inter = interleave() if interleave is not None else None
                for blk in range(NBLK):
                    if inter is not None:
                        next(inter, None)
                    t = in_pool.tile(
                        [128, 4, 4, W], BF16, tag=f"in{(b * NBLK + blk) % 8}"
                    )
                    nc.sync.dma_start(
                        t[:],
                        v[b, blk * 8 : blk * 8 + 8, :, :].rearrange(
                            "(i2 ic) (q hh) w -> (ic q) i2 hh w", i2=4, ic=2, q=64, hh=4
                        ),
                    )
                    s_blk = sblk_pool.tile([4, 4, 2, 512], BF16, tag="sblk")
                    for i2 in range(4):
                        p = psum_p.tile([4, 2, 512], F32, tag="P")
                        for s in range(2):
                            nc.tensor.matmul(
                                p[:, s, :],
                                lhsT=sb_y4cat,
                                rhs=t[:, i2, 2 * s : 2 * s + 2, :],
                                start=True,
                                stop=True,
                            )
                        dst = s_blk[:, i2, :, :]
                        if i2 % 2 == 0:
                            nc.vector.tensor_copy(dst, p[:])
                        else:
                            nc.scalar.copy(dst, p[:])
                    # bounce out: dscr[4*blk + i2, r, s, f] = s_blk[r, i2, s, f]
                    nc.gpsimd.dma_start(
                        dscr[4 * blk : 4 * blk + 4].rearrange(
                            "i2 r s f -> r i2 s f"
                        ),
                        s_blk[:],
                    )
                    if blk == 3:
                        # first-half readback + fused reductions overlap blocks 4-7
                        hp3 = tc.high_priority()
                        hp3.__enter__()
                        nc.gpsimd.dma_start(
                            s2[0:64].rearrange("p hh w -> p (hh w)"),
                            dscr[0:16].rearrange("ihi r s f -> (ihi r) (s f)"),
                        )
                        prodA = scratch.tile([128, 4, W], BF16, tag="prod")
                        nc.vector.scalar_tensor_tensor(
                            out=prodA[0:64], in0=s2[0:64], scalar=1.0,
                            in1=sb_wty[0:64], op0=MULT, op1=MULT,
                            accum_out=gy[0:64],
                        )
                        nc.vector.scalar_tensor_tensor(
                            out=prodA[64:128], in0=s2[0:64], scalar=1.0,
                            in1=sb_wtx[0:64], op0=MULT, op1=MULT,
                            accum_out=gx[0:64],
                        )
                        hp3.__exit__(None, None, None)
                # second-half readback + fused reductions
                hp7 = tc.high_priority()
                hp7.__enter__()
                nc.gpsimd.dma_start(
                    s2[64:128].rearrange("p hh w -> p (hh w)"),
                    dscr[16:32].rearrange("ihi r s f -> (ihi r) (s f)"),
                )
                prodB = scratch.tile([128, 4, W], BF16, tag="prod2")
                nc.vector.scalar_tensor_tensor(
                    out=prodB[0:64], in0=s2[64:128], scalar=1.0,
                    in1=sb_wty[64:128], op0=MULT, op1=MULT,
                    accum_out=gy[64:128],
                )
                nc.vector.scalar_tensor_tensor(
                    out=prodB[64:128], in0=s2[64:128], scalar=1.0,
                    in1=sb_wtx[64:128], op0=MULT, op1=MULT,
                    accum_out=gx[64:128],
                )
                hp7.__exit__(None, None, None)
                return gy, gx

            def tiny(b, gy, gx):
                """gy/gx -> ABcols [128, 32, 2] (A,B per ch-pair) + biasT [128, 4, 32]."""
                hp = tc.high_priority()
                hp.__enter__()
                ab_ps = psum_t.tile([1, 2 * CO], F32, tag="tiny")
                nc.tensor.matmul(
                    ab_ps[:], lhsT=gy[:], rhs=sb_my, start=True, stop=False
                )
                nc.tensor.matmul(
                    ab_ps[:], lhsT=gx[:], rhs=sb_mx, start=False, stop=True
                )
                ab_row = scratch.tile([1, 2 * CO], F32, tag="ti3")
                nc.vector.tensor_copy(ab_row[:], ab_ps[:])

                # ABcols[p, t, j] = (A,B)[2t + p//64, j]: two rank-1 outer
                # products (indicator-row x ab-row) accumulated on PE
                e_ps = psum_bc.tile([128, 32, 2], F32, tag="bc")
                nc.tensor.matmul(
                    e_ps[:], lhsT=sb_erow0, rhs=ab_row[:, 0:CO],
                    start=True, stop=False,
                )
                nc.tensor.matmul(
                    e_ps[:], lhsT=sb_erow1, rhs=ab_row[:, CO : 2 * CO],
                    start=False, stop=True,
                )
                abcols = bc_pool.tile([128, 32, 2], F32, tag="abcols")
                nc.vector.tensor_copy(abcols[:], e_ps[:])
                hp.__exit__(None, None, None)
                return (abcols,)

            def gen_stream(b, abcols):
                eng = 0
                for tg in range(NTG):
                    yield
                    ot = out_pool.tile([128, 4, 4, W], BF16, tag="out")
                    for tl in range(4):
                        ti = 4 * tg + tl
                        a_ap = abcols[:, ti, 0:1]
                        b_ap = abcols[:, ti, 1:2]
                        # xB = x_w * B on ACT; u-slice = ybig*A + xB fused
                        xb = bc_pool.tile([128, W], BF16, tag="xb")
                        nc.scalar.activation(
                            xb[:], sb_xrep,
                            mybir.ActivationFunctionType.Identity,
                            scale=b_ap,
                        )
                        xb_b = xb[:].rearrange("p (o w) -> p o w", o=1).broadcast_to(
                            (128, 4, W)
                        )
                        dst = ot[:, tl, :, :]
                        if eng % 4 != 3:
                            nc.vector.scalar_tensor_tensor(
                                out=dst, in0=sb_ybig, scalar=a_ap, in1=xb_b,
                                op0=MULT, op1=ADD,
                            )
                        else:
                            tmp = bc_pool.tile([128, 4, W], BF16, tag="gtmp")
                            nc.gpsimd.tensor_scalar(
                                out=tmp[:], in0=sb_ybig, scalar1=a_ap,
                                scalar2=None, op0=MULT,
                            )
                            nc.gpsimd.tensor_tensor(
                                out=dst, in0=tmp[:], in1=xb_b, op=ADD,
                            )
                        eng += 1
                    nc.scalar.dma_start(
                        u[b, tg * 8 : tg * 8 + 8, :, :].rearrange(
                            "(tl ic) (q hh) w -> (ic q) tl hh w", tl=4, ic=2, q=64, hh=4
                        ),
                        ot[:],
                    )

            gy0, gx0 = reduce_phase(0)
            ab0 = tiny(0, gy0, gx0)
            g0 = gen_stream(0, *ab0)
            gy1, gx1 = reduce_phase(1, interleave=lambda: g0)
            for _ in g0:
                pass
            ab1 = tiny(1, gy1, gx1)
            for _ in gen_stream(1, *ab1):
                pass

    nc.compile()
    return nc


def make_in_maps(v, psi, phi):
    y = np.linspace(-1.0, 1.0, H, dtype=np.float32)
    x = np.linspace(-1.0, 1.0, W, dtype=np.float32)
    dx = np.float32(2.0 / (W - 1))
    dy = np.float32(2.0 / (H - 1))

    q = np.arange(64)
    # stationary for the h-quad reduction: block-diagonal by channel parity
    # cols: [y4*even, 1*even, y4*odd, 1*odd]; partition p = 64*ic + q
    y4cat = np.zeros((128, 4), np.float32)
    y4cat[0:64, 0] = y[4 * q]
    y4cat[0:64, 1] = 1.0
    y4cat[64:128, 2] = y[4 * q]
    y4cat[64:128, 3] = 1.0

    # s2 partition layout: P = 4*ihi + r, r = 2*ic + role, i = 2*ihi + ic
    # role 0 rows hold y4-weighted sums (weight 1); role 1 rows hold per-hh
    # colsums (Vy correction dy*hh; Vx weight x_w)
    P = np.arange(128)
    role = P % 2
    ic = (P % 4) // 2
    i_of_p = 2 * (P // 4) + ic
    wty = np.zeros((128, 4, W), np.float32)
    wtx = np.zeros((128, 4, W), np.float32)
    wty[role == 0, :, :] = 1.0
    wty[role == 1, :, :] = (dy * np.arange(4, dtype=np.float32))[None, :, None]
    wtx[role == 1, :, :] = x[None, None, :]

    # gy/gx -> inner: psi packs indexed by the same P layout, then folded
    # through phi on the host: m{y,x}{c}[P, 2t+j] = sum_r psi_pack[P, r] *
    # phi[2t+c, r, j], so ab_c = gy^T @ my_c + gx^T @ mx_c directly
    psiy_p = (dx * psi[:, i_of_p, 0].T).astype(np.float32)
    psix_p = (dx * psi[:, i_of_p, 1].T).astype(np.float32)
    psix_p[role == 0, :] = 0.0

    # inner -> (A,B) interleaved per channel pair: phip_c[r, 2t+j] for o=2t+c
    t_idx = np.arange(32)
    phip0 = np.zeros((R, CO), np.float32)
    phip1 = np.zeros((R, CO), np.float32)
    for j in range(2):
        phip0[:, 2 * t_idx + j] = phi[2 * t_idx, :, j].T
        phip1[:, 2 * t_idx + j] = phi[2 * t_idx + 1, :, j].T

    erow0 = np.zeros((1, 128), np.float32)
    erow0[0, 0:64] = 1.0
    erow1 = np.zeros((1, 128), np.float32)
    erow1[0, 64:128] = 1.0

    ycol4 = np.empty((128, 4), np.float32)
    for hh in range(4):
        ycol4[0:64, hh] = y[4 * q + hh]
        ycol4[64:128, hh] = y[4 * q + hh]

    shards = np.ascontiguousarray(
        v.reshape(N_CORES, BPC, CI, H, W).astype(BFNP)
    )
    ybig = np.empty((128, 4, W), np.float32)
    for hh in range(4):
        ybig[0:64, hh, :] = y[4 * q + hh][:, None]
        ybig[64:128, hh, :] = y[4 * q + hh][:, None]
    catbf = np.concatenate(
        [
            y4cat,
            wty.reshape(128, 4 * W),
            wtx.reshape(128, 4 * W),
            np.broadcast_to(x, (128, W)),
            ybig.reshape(128, 4 * W),
        ],
        axis=1,
    ).astype(BFNP)
    catf32 = np.concatenate(
        [psiy_p @ phip0, psiy_p @ phip1, psix_p @ phip0, psix_p @ phip1, ycol4],
        axis=1,
    ).astype(np.float32)
    common = {
        "catbf": catbf,
        "catf32": catf32,
        "erows": np.concatenate([erow0, erow1], axis=1),
    }
    return [{"v": shards[i], **common} for i in range(N_CORES)]


_NC_CACHE = None


def kernel(v, psi, phi):
    global _NC_CACHE
    if _NC_CACHE is None:
        _NC_CACHE = build_nc()
    nc = _NC_CACHE
    in_maps = make_in_maps(
        np.ascontiguousarray(v, dtype=np.float32),
        np.asarray(psi, dtype=np.float32),
        np.asarray(phi, dtype=np.float32),
    )
    res = run_bass_kernel_spmd(nc, in_maps, core_ids=list(range(N_CORES)))
    return np.concatenate(
        [r["u"].astype(np.float32) for r in res.results], axis=0
    )


if __name__ == "__main__":
    build_nc()
    print("build ok")


# revision 56
# speedup vs baseline: 2.6601x; 2.6601x over previous
"""Trainium2 Bass kernel for the low-rank linear operator.

Math: the reference collapses algebraically. With y = linspace(-1,1,H),
x = linspace(-1,1,W), dx = 2/(W-1):

  Vy[b,i] = sum_{h,w} v[b,i,h,w] * y_h
  Vx[b,i] = sum_{h,w} v[b,i,h,w] * x_w
  inner[b,r] = dx * sum_i (Vy[b,i]*psi[r,i,0] + Vx[b,i]*psi[r,i,1])
  A[b,o] = sum_r inner[b,r]*phi[o,r,0];  Bc[b,o] = sum_r inner[b,r]*phi[o,r,1]
  u[b,o,h,w] = A[b,o]*y_h + Bc[b,o]*x_w

Sharding: data-parallel over batch, 2 batches per core, 8 cores, no
collectives.

The kernel is HBM-bandwidth bound (read v + write u). All HBM traffic is
bf16 (host casts in/out; rel-err budget 2e-2 vs ~0.5% incurred), halving
bytes vs f32. Layout: four h-rows per partition, p = 64*(i%2) + h//4, so
every DMA descriptor moves 2KB contiguous even at 2B/elem.

Reduction: PE matmul with a block-diagonal [128,4] stationary reduces the
h-quad partition dim for both channel parities at once -> psum [4, 512]
rows (even-ch y-weighted, even-ch colsum, odd-ch y-weighted, odd-ch
colsum). Since y is affine, y[4q+hh] = y[4q] + hh*dy, the hh correction is
folded into a second full-width DVE mult+reduce pass after an SBUF->SBUF
DMA re-partitions the drained psum rows to [128, (hh w)]. Tiny PE matmuls
give inner -> (A,B) -> per-partition scale/bias; DVE/ACT/Pool generate u
tiles as x_w*B + (A*y) bias directly in bf16.
"""

import sys

try:
    import concourse.bass as bass  # noqa: F401
except ImportError:
    for _p in ("/opt/trn_rl_repo", "/root/.axon_site/_ro/trn_rl_repo"):
        if _p not in sys.path:
            sys.path.insert(0, _p)

import ml_dtypes
import numpy as np

import concourse.bacc as bacc
import concourse.bass as bass
import concourse.mybir as mybir
import concourse.tile as tile
from concourse.bass_utils import run_bass_kernel_spmd

F32 = mybir.dt.float32
BF16 = mybir.dt.bfloat16
MULT = mybir.AluOpType.mult
ADD = mybir.AluOpType.add
BFNP = ml_dtypes.bfloat16

B, CI, CO, R, H, W = 16, 64, 64, 64, 256, 256
N_CORES = 8
BPC = B // N_CORES  # batches per core
NBLK = 8            # input DMA blocks per batch (8 channels each)
NTG = 8             # output DMA blocks per batch (8 channels each)

# generation-engine rotation (gpsimd has no drain duty -> give it more)
_GEN_ENGINES = (
    "dve", "act", "pool", "dve", "act", "dve", "pool", "act",
    "dve", "act", "pool", "dve", "act", "dve", "pool", "act",
)


def build_nc():
    nc = bacc.Bacc("TRN2", target_bir_lowering=False, debug=False)

    v = nc.dram_tensor("v", [BPC, CI, H, W], BF16, kind="ExternalInput")
    catbf = nc.dram_tensor("catbf", [128, 4 + 3 * 4 * W + W], BF16, kind="ExternalInput")
    catf32 = nc.dram_tensor("catf32", [128, 4 * CO + 4], F32, kind="ExternalInput")
    erows = nc.dram_tensor("erows", [1, 256], F32, kind="ExternalInput")
    u = nc.dram_tensor("u", [BPC, CO, H, W], BF16, kind="ExternalOutput")

    with tile.TileContext(nc) as tc:
        with (
            tc.tile_pool(name="consts", bufs=1) as consts,
            tc.tile_pool(name="inp", bufs=1) as in_pool,
            tc.tile_pool(name="outp", bufs=3) as out_pool,
            tc.tile_pool(name="scr", bufs=1) as scratch,
            tc.tile_pool(name="sblkp", bufs=4) as sblk_pool,
            tc.tile_pool(name="s2p", bufs=1) as s2_pool,
            tc.tile_pool(name="bc", bufs=4) as bc_pool,
            tc.tile_pool(name="psumP", bufs=3, space="PSUM") as psum_p,
            tc.tile_pool(name="psumT", bufs=1, space="PSUM") as psum_t,
            tc.tile_pool(name="psumBC", bufs=1, space="PSUM") as psum_bc,
            tc.tile_pool(name="dram", bufs=2, space="DRAM") as dram_pool,
        ):
            sb_catbf = consts.tile([128, 4 + 3 * 4 * W + W], BF16)
            nc.scalar.dma_start(sb_catbf[:], catbf[:])
            sb_catf = consts.tile([128, 4 * CO + 4], F32)
            nc.scalar.dma_start(sb_catf[:], catf32[:])
            sb_erows = consts.tile([1, 256], F32)
            nc.scalar.dma_start(sb_erows[:], erows[:])
            sb_y4cat = sb_catbf[:, 0:4]
            sb_wty = sb_catbf[:, 4 : 4 + 4 * W].rearrange("p (hh w) -> p hh w", hh=4)
            sb_wtx = sb_catbf[:, 4 + 4 * W : 4 + 8 * W].rearrange(
                "p (hh w) -> p hh w", hh=4
            )
            sb_xrep = sb_catbf[:, 4 + 8 * W : 4 + 9 * W]
            sb_ybig = sb_catbf[:, 4 + 9 * W : 4 + 13 * W].rearrange(
                "p (hh w) -> p hh w", hh=4
            )
            sb_my = sb_catf[:, 0 : 2 * CO]
            sb_mx = sb_catf[:, 2 * CO : 4 * CO]
            sb_ycol4 = sb_catf[:, 4 * CO : 4 * CO + 4]
            sb_erow0 = sb_erows[:, 0:128]
            sb_erow1 = sb_erows[:, 128:256]

            def reduce_phase(b, interleave=None):
                """v[b] -> s2[b]: [128=(ihi r), (hh w)] partial sums."""
                s2 = s2_pool.tile([128, 4, W], BF16, tag=f"s2{b}")
                gy = scratch.tile([128, 1], F32, tag=f"gy{b}")
                gx = scratch.tile([128, 1], F32, tag=f"gx{b}")
                dscr = dram_pool.tile([32, 4, 2, 512], BF16, tag="dscr")
                inter = interleave() if interleave is not None else None
                for blk in range(NBLK):
                    if inter is not None:
                        next(inter, None)
                    t = in_pool.tile(
                        [128, 4, 4, W], BF16, tag=f"in{(b * NBLK + blk) % 8}"
                    )
                    nc.sync.dma_start(
                        t[:],
                        v[b, blk * 8 : blk * 8 + 8, :, :].rearrange(
                            "(i2 ic) (q hh) w -> (ic q) i2 hh w", i2=4, ic=2, q=64, hh=4
                        ),
                    )
                    s_blk = sblk_pool.tile([4, 4, 2, 512], BF16, tag="sblk")
                    for i2 in range(4):
                        p = psum_p.tile([4, 2, 512], F32, tag="P")
                        for s in range(2):
                            nc.tensor.matmul(
                                p[:, s, :],
                                lhsT=sb_y4cat,
                                rhs=t[:, i2, 2 * s : 2 * s + 2, :],
                                start=True,
                                stop=True,
                            )
                        dst = s_blk[:, i2, :, :]
                        if i2 % 2 == 0:
                            nc.vector.tensor_copy(dst, p[:])
                        else:
                            nc.scalar.copy(dst, p[:])
                    # bounce out: dscr[4*blk + i2, r, s, f] = s_blk[r, i2, s, f]
                    nc.gpsimd.dma_start(
                        dscr[4 * blk : 4 * blk + 4].rearrange(
                            "i2 r s f -> r i2 s f"
                        ),
                        s_blk[:],
                    )
                    if blk == 3:
                        # first-half readback + fused reductions overlap blocks 4-7
                        hp3 = tc.high_priority()
                        hp3.__enter__()
                        nc.gpsimd.dma_start(
                            s2[0:64].rearrange("p hh w -> p (hh w)"),
                            dscr[0:16].rearrange("ihi r s f -> (ihi r) (s f)"),
                        )
                        prodA = scratch.tile([128, 4, W], BF16, tag="prod")
                        nc.vector.scalar_tensor_tensor(
                            out=prodA[0:64], in0=s2[0:64], scalar=1.0,
                            in1=sb_wty[0:64], op0=MULT, op1=MULT,
                            accum_out=gy[0:64],
                        )
                        nc.vector.scalar_tensor_tensor(
                            out=prodA[64:128], in0=s2[0:64], scalar=1.0,
                            in1=sb_wtx[0:64], op0=MULT, op1=MULT,
                            accum_out=gx[0:64],
                        )
                        hp3.__exit__(None, None, None)
                # second-half readback + fused reductions
                hp7 = tc.high_priority()
                hp7.__enter__()
                nc.gpsimd.dma_start(
                    s2[64:128].rearrange("p hh w -> p (hh w)"),
                    dscr[16:32].rearrange("ihi r s f -> (ihi r) (s f)"),
                )
                prodB = scratch.tile([128, 4, W], BF16, tag="prod2")
                nc.vector.scalar_tensor_tensor(
                    out=prodB[0:64], in0=s2[64:128], scalar=1.0,
                    in1=sb_wty[64:128], op0=MULT, op1=MULT,
                    accum_out=gy[64:128],
                )
                nc.vector.scalar_tensor_tensor(
                    out=prodB[64:128], in0=s2[64:128], scalar=1.0,
                    in1=sb_wtx[64:128], op0=MULT, op1=MULT,
                    accum_out=gx[64:128],
                )
                hp7.__exit__(None, None, None)
                return gy, gx

            def tiny(b, gy, gx):
                """gy/gx -> ABcols [128, 32, 2] (A,B per ch-pair) + biasT [128, 4, 32]."""
                hp = tc.high_priority()
                hp.__enter__()
                ab_ps = psum_t.tile([1, 2 * CO], F32, tag="tiny")
                nc.tensor.matmul(
                    ab_ps[:], lhsT=gy[:], rhs=sb_my, start=True, stop=False
                )
                nc.tensor.matmul(
                    ab_ps[:], lhsT=gx[:], rhs=sb_mx, start=False, stop=True
                )
                ab_row = scratch.tile([1, 2 * CO], F32, tag="ti3")
                nc.vector.tensor_copy(ab_row[:], ab_ps[:])

                # ABcols[p, t, j] = (A,B)[2t + p//64, j]: two rank-1 outer
                # products (indicator-row x ab-row) accumulated on PE
                e_ps = psum_bc.tile([128, 32, 2], F32, tag="bc")
                nc.tensor.matmul(
                    e_ps[:], lhsT=sb_erow0, rhs=ab_row[:, 0:CO],
                    start=True, stop=False,
                )
                nc.tensor.matmul(
                    e_ps[:], lhsT=sb_erow1, rhs=ab_row[:, CO : 2 * CO],
                    start=False, stop=True,
                )
                abcols = bc_pool.tile([128, 32, 2], F32, tag="abcols")
                nc.vector.tensor_copy(abcols[:], e_ps[:])
                biast = bc_pool.tile([128, 4, 32], F32, tag="biast")
                for hh in range(4):
                    nc.vector.tensor_scalar(
                        out=biast[:, hh, :], in0=abcols[:, :, 0],
                        scalar1=sb_ycol4[:, hh : hh + 1], scalar2=None, op0=MULT,
                    )
                hp.__exit__(None, None, None)
                return abcols, biast

            def gen_stream(b, abcols, biast):
                eng = 0
                for tg in range(NTG):
                    yield
                    ot = out_pool.tile([128, 4, 4, W], BF16, tag="out")
                    for tl in range(4):
                        ti = 4 * tg + tl
                        sc_ap = abcols[:, ti, 1:2]
                        for hh in range(4):
                            bias_ap = biast[:, hh, ti : ti + 1]
                            dst = ot[:, tl, hh, :]
                            which = _GEN_ENGINES[eng % len(_GEN_ENGINES)]
                            eng += 1
                            if which == "dve":
                                nc.vector.tensor_scalar(
                                    out=dst, in0=sb_xrep, scalar1=sc_ap,
                                    scalar2=bias_ap, op0=MULT, op1=ADD,
                                )
                            elif which == "pool":
                                nc.gpsimd.tensor_scalar(
                                    out=dst, in0=sb_xrep, scalar1=sc_ap,
                                    scalar2=bias_ap, op0=MULT, op1=ADD,
                                )
                            else:
                                nc.scalar.activation(
                                    dst, sb_xrep,
                                    mybir.ActivationFunctionType.Identity,
                                    bias=bias_ap, scale=sc_ap,
                                )
                    nc.scalar.dma_start(
                        u[b, tg * 8 : tg * 8 + 8, :, :].rearrange(
                            "(tl ic) (q hh) w -> (ic q) tl hh w", tl=4, ic=2, q=64, hh=4
                        ),
                        ot[:],
                    )

            gy0, gx0 = reduce_phase(0)
            ab0 = tiny(0, gy0, gx0)
            g0 = gen_stream(0, *ab0)
            gy1, gx1 = reduce_phase(1, interleave=lambda: g0)
            for _ in g0:
                pass
            ab1 = tiny(1, gy1, gx1)
            for _ in gen_stream(1, *ab1):
                pass

    nc.compile()
    return nc


def make_in_maps(v, psi, phi):
    y = np.linspace(-1.0, 1.0, H, dtype=np.float32)
    x = np.linspace(-1.0, 1.0, W, dtype=np.float32)
    dx = np.float32(2.0 / (W - 1))
    dy = np.float32(2.0 / (H - 1))

    q = np.arange(64)
    # stationary for the h-quad reduction: block-diagonal by channel parity
    # cols: [y4*even, 1*even, y4*odd, 1*odd]; partition p = 64*ic + q
    y4cat = np.zeros((128, 4), np.float32)
    y4cat[0:64, 0] = y[4 * q]
    y4cat[0:64, 1] = 1.0
    y4cat[64:128, 2] = y[4 * q]
    y4cat[64:128, 3] = 1.0

    # s2 partition layout: P = 4*ihi + r, r = 2*ic + role, i = 2*ihi + ic
    # role 0 rows hold y4-weighted sums (weight 1); role 1 rows hold per-hh
    # colsums (Vy correction dy*hh; Vx weight x_w)
    P = np.arange(128)
    role = P % 2
    ic = (P % 4) // 2
    i_of_p = 2 * (P // 4) + ic
    wty = np.zeros((128, 4, W), np.float32)
    wtx = np.zeros((128, 4, W), np.float32)
    wty[role == 0, :, :] = 1.0
    wty[role == 1, :, :] = (dy * np.arange(4, dtype=np.float32))[None, :, None]
    wtx[role == 1, :, :] = x[None, None, :]

    # gy/gx -> inner: psi packs indexed by the same P layout, then folded
    # through phi on the host: m{y,x}{c}[P, 2t+j] = sum_r psi_pack[P, r] *
    # phi[2t+c, r, j], so ab_c = gy^T @ my_c + gx^T @ mx_c directly
    psiy_p = (dx * psi[:, i_of_p, 0].T).astype(np.float32)
    psix_p = (dx * psi[:, i_of_p, 1].T).astype(np.float32)
    psix_p[role == 0, :] = 0.0

    # inner -> (A,B) interleaved per channel pair: phip_c[r, 2t+j] for o=2t+c
    t_idx = np.arange(32)
    phip0 = np.zeros((R, CO), np.float32)
    phip1 = np.zeros((R, CO), np.float32)
    for j in range(2):
        phip0[:, 2 * t_idx + j] = phi[2 * t_idx, :, j].T
        phip1[:, 2 * t_idx + j] = phi[2 * t_idx + 1, :, j].T

    erow0 = np.zeros((1, 128), np.float32)
    erow0[0, 0:64] = 1.0
    erow1 = np.zeros((1, 128), np.float32)
    erow1[0, 64:128] = 1.0

    ycol4 = np.empty((128, 4), np.float32)
    for hh in range(4):
        ycol4[0:64, hh] = y[4 * q + hh]
        ycol4[64:128, hh] = y[4 * q + hh]

    shards = np.ascontiguousarray(
        v.reshape(N_CORES, BPC, CI, H, W).astype(BFNP)
    )
    ybig = np.empty((128, 4, W), np.float32)
    for hh in range(4):
        ybig[0:64, hh, :] = y[4 * q + hh][:, None]
        ybig[64:128, hh, :] = y[4 * q + hh][:, None]
    catbf = np.concatenate(
        [
            y4cat,
            wty.reshape(128, 4 * W),
            wtx.reshape(128, 4 * W),
            np.broadcast_to(x, (128, W)),
            ybig.reshape(128, 4 * W),
        ],
        axis=1,
    ).astype(BFNP)
    catf32 = np.concatenate(
        [psiy_p @ phip0, psiy_p @ phip1, psix_p @ phip0, psix_p @ phip1, ycol4],
        axis=1,
    ).astype(np.float32)
    common = {
        "catbf": catbf,
        "catf32": catf32,
        "erows": np.concatenate([erow0, erow1], axis=1),
    }
    return [{"v": shards[i], **common} for i in range(N_CORES)]


_NC_CACHE = None


def kernel(v, psi, phi):
    global _NC_CACHE
    if _NC_CACHE is None:
        _NC_CACHE = build_nc()
    nc = _NC_CACHE
    in_maps = make_in_maps(
        np.ascontiguousarray(v, dtype=np.float32),
        np.asarray(psi, dtype=np.float32),
        np.asarray(phi, dtype=np.float32),
    )
    res = run_bass_kernel_spmd(nc, in_maps, core_ids=list(range(N_CORES)))
    return np.concatenate(
        [r["u"].astype(np.float32) for r in res.results], axis=0
    )


if __name__ == "__main__":
    build_nc()
    print("build ok")
